# revision 12
# baseline (speedup 1.0000x reference)
"""Trainium2 Bass kernel for the fused attention+LN+GELU+projection module.

Shapes (hardcoded): x [B=256, S=512, D=512]; k/q/v_w [H=256, D]; attn_bias [S, H];
out_w [D, S*H]; output [B, 1, D].

Distribution across 8 NeuronCores (v2, bf16):
 - phases 1-7 (QKV proj, scores, softmax, apply, +bias, LN, GELU): data-parallel
   over batch, 32 batches/core, all compute in bf16 (fp32 PSUM accumulate).
 - activations stream out in 4 AllToAll chunks (8 batches/group) overlapped
   with attention compute; phase 8 (y = act @ out_w.T) reads the redistributed
   activations via XBAR transpose-DMA (no PE transposes), accumulates partial
   products over this core's 1/8 slice of the contraction dim, and a
   ReduceScatter leaves each core with its 32-row shard of the output.
"""

import sys

sys.path.insert(0, "/opt/trn_rl_repo")

import ml_dtypes
import numpy as np

import concourse.bacc as bacc
import concourse.tile as tile
from concourse import mybir
from concourse.bass_utils import run_bass_kernel_spmd
from concourse.hw_specs import get_activation_tables
from concourse.tile_rust import add_dep_helper
import bass_rust as _bass_rust

N_CORES = 8
B, S, H, D = 256, 512, 256, 512
NB = B // N_CORES          # batches per core
SCALE = 1.0 / (B ** 0.5)   # score scale (batch-size based, faithful to ref)
LN_EPS = 1e-5
NDT = D // 128             # 4 d-tiles
NST = S // 128             # 4 s-tiles
NHT = H // 128             # 2 h-tiles
SLICE = (S // N_CORES) * H  # 16384 contraction elems per core
NC_T = SLICE // 128        # 128 contraction tiles per core
G = 8                      # batches per ACT-table/A2A group
NCH = NB // G              # 4 A2A chunks

BF16 = mybir.dt.bfloat16
F32 = mybir.dt.float32
AF = mybir.ActivationFunctionType
ALU = mybir.AluOpType
BFNP = ml_dtypes.bfloat16


class _Bacc(bacc.Bacc):
    """Bacc whose activation-table binding is restricted so that exp/ln are
    only servable by natural_log_exp_and_others and gelu by gelu_and_others.
    Avoids per-op ACT_TABLE_LOAD thrash (~2.7us each) from the default
    first-match binding."""

    def insert_act_table_loads(self):
        has_activation = any(
            isinstance(i, mybir.InstActivation)
            for b in self.main_func.blocks
            for i in b.instructions
        )
        if not has_activation:
            return
        keep = {"natural_log_exp_and_others", "gelu_and_others"}
        strip = {AF.Exp, AF.Ln, AF.Gelu}
        tables = []
        for name, funcs in get_activation_tables(self.m.arch).items():
            if name not in keep:
                funcs = funcs - strip
            tables.append((name, funcs))
        _bass_rust.insert_act_table_loads(self, tables)


def _build(ln_trivial: bool):
    nc = _Bacc("TRN2", target_bir_lowering=False, debug=False,
               num_devices=N_CORES)

    # ---- DRAM I/O ----
    xT = nc.dram_tensor("xT", [NB, NDT, 128, S], BF16, kind="ExternalInput").ap()
    kq_wT = nc.dram_tensor("kq_wT", [NDT, 128, 2 * H], BF16, kind="ExternalInput").ap()
    v_wT = nc.dram_tensor("v_wT", [NDT, 128, H], BF16, kind="ExternalInput").ap()
    kqb_row = nc.dram_tensor("kqb_row", [1, 2 * H], BF16, kind="ExternalInput").ap()
    v_b2 = nc.dram_tensor("v_b2", [NHT, 128, 1], F32, kind="ExternalInput").ap()
    ab = nc.dram_tensor("ab", [128, NST * H], BF16, kind="ExternalInput").ap()
    outb8 = nc.dram_tensor("outb8", [128, D], F32, kind="ExternalInput").ap()
    ones128 = nc.dram_tensor("ones128", [128, 128], BF16, kind="ExternalInput").ap()
    owT = nc.dram_tensor("owT", [NC_T, 128, D], BF16, kind="ExternalInput").ap()
    if not ln_trivial:
        lng = nc.dram_tensor("lng", [128, H], BF16, kind="ExternalInput").ap()
        lnb = nc.dram_tensor("lnb", [128, H], BF16, kind="ExternalInput").ap()
    y_out = nc.dram_tensor("y", [NB, D], F32, kind="ExternalOutput").ap()

    # internal DRAM (collective buffers)
    a2a_in = [
        nc.dram_tensor(f"a2a_in{g}", [N_CORES, G, S // N_CORES, H], BF16).ap()
        for g in range(NCH)
    ]
    a2a_out = nc.dram_tensor("a2a_out", [N_CORES * NB, SLICE], BF16).ap()
    y_part = nc.dram_tensor("y_part", [N_CORES * NB, D], F32).ap()
    y_red = nc.dram_tensor("y_red", [NB, D], F32).ap()

    rg = [list(range(N_CORES))]

    with tile.TileContext(nc) as tc:
        with (
            tc.tile_pool(name="const", bufs=1) as constp,
            tc.tile_pool(name="xt", bufs=12) as xtp,
            tc.tile_pool(name="kqsb", bufs=3) as kqp,
            tc.tile_pool(name="vtsb", bufs=3) as vtp,
            tc.tile_pool(name="esb", bufs=3) as ep,
            tc.tile_pool(name="wsb", bufs=3) as wp,
            tc.tile_pool(name="sden", bufs=3) as sdp,
            tc.tile_pool(name="tsb", bufs=G + 3) as tp,
            tc.tile_pool(name="sqsb", bufs=3) as sqp,
            tc.tile_pool(name="actsb", bufs=8) as actp,
            tc.tile_pool(name="stat", bufs=4 * G + 8) as statp,
            tc.tile_pool(name="tmp4", bufs=12) as tmpp,
        ):
            # ---- persistent constants ----
            kqw_sb = []
            vw_sb = []
            for dt_ in range(NDT):
                t = constp.tile([128, 2 * H], BF16, tag=f"kqw{dt_}")
                nc.sync.dma_start(t[:], kq_wT[dt_])
                kqw_sb.append(t)
                t = constp.tile([128, H], BF16, tag=f"vw{dt_}")
                nc.sync.dma_start(t[:], v_wT[dt_])
                vw_sb.append(t)
            kqb_sb = constp.tile([1, 2 * H], BF16, tag="kqb")
            nc.sync.dma_start(kqb_sb[:], kqb_row[:])
            vb_sb = []
            for ht in range(NHT):
                t = constp.tile([128, 1], F32, tag=f"vb{ht}")
                nc.sync.dma_start(t[:], v_b2[ht])
                vb_sb.append(t)
            ab_sb = constp.tile([128, NST * H], BF16, tag="ab")
            nc.sync.dma_start(ab_sb[:], ab[:])
            outb_sb = constp.tile([128, D], F32, tag="outb")
            nc.sync.dma_start(outb_sb[:], outb8[:])
            if not ln_trivial:
                lng_sb = constp.tile([128, H], BF16, tag="lng")
                nc.sync.dma_start(lng_sb[:], lng[:])
                lnb_sb = constp.tile([128, H], BF16, tag="lnb")
                nc.sync.dma_start(lnb_sb[:], lnb[:])
            ones_sb = constp.tile([128, 128], BF16, tag="ones")
            nc.sync.dma_start(ones_sb[:], ones128[:])
            ones_col = ones_sb[:, 0:1]   # [128, 1] lhsT for partition sums
            ones_row = ones_sb[0:1, :]   # [1, 128] lhsT for broadcasts
            eps_sb = constp.tile([128, 1], F32, tag="eps")
            nc.gpsimd.memset(eps_sb[:], LN_EPS)

            # ---- per-batch attention pipeline ----
            # PSUM budget (8 banks): vt 2 + kq 2x2 + sc 1 + out5 2 = 8
            with (
                tc.tile_pool(name="vtps", bufs=1, space="PSUM") as vtps,
                tc.tile_pool(name="kqps", bufs=2, space="PSUM") as kqpsp,
                tc.tile_pool(name="scps", bufs=1, space="PSUM") as scp,
                tc.tile_pool(name="o5ps", bufs=1, space="PSUM") as o5p,
            ):
                pend = []            # deferred-GELU state per batch in group
                grp_tbl_insts = []   # this group's exp/ln ACT instructions
                prev_gelu = None     # last gelu instruction of previous group
                for b in range(NB):
                    ch = b // G      # a2a chunk this batch belongs to
                    bb = b % G
                    xt = []
                    for dt_ in range(NDT):
                        t = xtp.tile([128, S], BF16, tag="xt")
                        nc.sync.dma_start(t[:], xT[b, dt_])
                        xt.append(t)

                    # vT[h, s] = sum_d v_wT[d, h] * xT[d, s]  (+v_b per-part)
                    vt_sb = vtp.tile([128, 2 * S], BF16, tag="vt")
                    for ht in range(NHT):
                        vps = vtps.tile([128, S], F32, tag="vtps")
                        for dt_ in range(NDT):
                            nc.tensor.matmul(
                                vps[:],
                                vw_sb[dt_][:, ht * 128:(ht + 1) * 128],
                                xt[dt_][:],
                                start=(dt_ == 0), stop=(dt_ == NDT - 1))
                        nc.scalar.activation(
                            vt_sb[:, ht * S:(ht + 1) * S], vps[:],
                            AF.Identity, bias=vb_sb[ht][:])

                    # kq[s, j] = sum_d x[s, d] * [k_wT | q_wT][d, j] + bias
                    # (bias folded in as a K=1 matmul with a ones column)
                    kq_sb = kqp.tile([128, 4 * 512], BF16, tag="kq")
                    for sp in range(2):
                        kqps = kqpsp.tile([128, 1024], F32, tag="kqps")
                        for half in range(2):
                            st = 2 * sp + half
                            c0 = half * 512
                            for dt_ in range(NDT):
                                nc.tensor.matmul(
                                    kqps[:, c0:c0 + 512],
                                    xt[dt_][:, st * 128:(st + 1) * 128],
                                    kqw_sb[dt_][:],
                                    start=(dt_ == 0), stop=False)
                            nc.tensor.matmul(
                                kqps[:, c0:c0 + 512],
                                ones_row, kqb_sb[:],
                                start=False, stop=True)
                        if sp == 0:
                            nc.scalar.copy(kq_sb[:, 0:1024], kqps[:])
                        else:
                            nc.vector.tensor_copy(kq_sb[:, 1024:2048], kqps[:])

                    # scores[h, g] = sum_s k[s, h] q[s, g]; both ht halves in
                    # one bank; e = exp(scores/16)
                    sc = scp.tile([128, 512], F32, tag="scps")
                    for ht in range(NHT):
                        for st in range(NST):
                            nc.tensor.matmul(
                                sc[:, ht * H:(ht + 1) * H],
                                kq_sb[:, st * 512 + ht * 128:
                                      st * 512 + (ht + 1) * 128],
                                kq_sb[:, st * 512 + H:(st + 1) * 512],
                                start=(st == 0), stop=(st == NST - 1))
                    e_sb = ep.tile([128, 512], BF16, tag="e")
                    ei = nc.scalar.activation(e_sb[:], sc[:], AF.Exp,
                                              scale=SCALE)
                    grp_tbl_insts.append(ei)

                    # softmax denominator: reuse the scores bank.
                    # sm[0, g] = sum_h e[h, g]  (ones-matmuls, accumulated)
                    for ht in range(NHT):
                        nc.tensor.matmul(sc[0:1, 0:H], ones_col,
                                         e_sb[:, ht * H:(ht + 1) * H],
                                         start=(ht == 0), stop=(ht == NHT - 1))
                    # rec = 1/denom = exp(-ln(denom)) on the scalar engine
                    # (both funcs live in the natural_log_exp table set)
                    lnd = sdp.tile([1, H], F32, tag="lnd")
                    ldi = nc.scalar.activation(lnd[:], sc[0:1, 0:H], AF.Ln)
                    grp_tbl_insts.append(ldi)
                    sden = sdp.tile([1, H], BF16, tag="sden")
                    rdi = nc.scalar.activation(sden[:], lnd[:], AF.Exp,
                                               scale=-1.0)
                    grp_tbl_insts.append(rdi)
                    # broadcast reciprocal to all partitions (both halves)
                    for ht in range(NHT):
                        nc.tensor.matmul(sc[:, ht * H:(ht + 1) * H],
                                         ones_row, sden[:],
                                         start=True, stop=True)
                    w_sb = wp.tile([128, 512], BF16, tag="w")
                    nc.vector.tensor_mul(w_sb[:], e_sb[:], sc[:])

                    # out5[s, g] = sum_h vT[h, s] w[h, g]; +attn_bias fused
                    # with the LN mean-sum via tensor_tensor_reduce
                    o5 = o5p.tile([128, 1024], F32, tag="o5ps")
                    for st in range(NST):
                        for ht in range(NHT):
                            nc.tensor.matmul(
                                o5[:, st * H:(st + 1) * H],
                                vt_sb[:, ht * S + st * 128:
                                      ht * S + (st + 1) * 128],
                                w_sb[:, ht * H:(ht + 1) * H],
                                start=(ht == 0), stop=(ht == NHT - 1))
                    t_sb = tp.tile([128, 1024], BF16, tag="t")
                    nc.vector.tensor_add(t_sb[:], o5[:], ab_sb[:])
                    sums = statp.tile([128, 4], F32, tag="sums")
                    nc.vector.tensor_reduce(
                        sums[:], t_sb[:].rearrange("p (g c) -> p g c", g=4),
                        axis=mybir.AxisListType.X, op=ALU.add)
                    sq_sb = sqp.tile([128, 1024], BF16, tag="sq")
                    nc.vector.tensor_mul(sq_sb[:], t_sb[:], t_sb[:])
                    sumsq = statp.tile([128, 4], F32, tag="sumsq")
                    nc.vector.tensor_reduce(
                        sumsq[:], sq_sb[:].rearrange("p (g c) -> p g c", g=4),
                        axis=mybir.AxisListType.X, op=ALU.add)
                    # rstd = (var+eps)^-0.5 = exp(-0.5*ln(var+eps))
                    mean = tmpp.tile([128, 4], F32, tag="mean")
                    nc.vector.tensor_scalar_mul(mean[:], sums[:], 1.0 / H)
                    msq = tmpp.tile([128, 4], F32, tag="msq")
                    nc.vector.tensor_mul(msq[:], mean[:], mean[:])
                    q2 = tmpp.tile([128, 4], F32, tag="q2")
                    nc.vector.tensor_scalar_mul(q2[:], sumsq[:], 1.0 / H)
                    var = tmpp.tile([128, 4], F32, tag="var")
                    nc.vector.tensor_sub(var[:], q2[:], msq[:])
                    lnv = tmpp.tile([128, 4], F32, tag="lnv")
                    li = nc.scalar.activation(lnv[:], var[:], AF.Ln,
                                              bias=eps_sb[:])
                    grp_tbl_insts.append(li)
                    rstd = statp.tile([128, 4], F32, tag="rstd")
                    ri = nc.scalar.activation(rstd[:], lnv[:], AF.Exp,
                                              scale=-0.5)
                    grp_tbl_insts.append(ri)
                    nbt = tmpp.tile([128, 4], F32, tag="nbt")
                    nc.vector.tensor_mul(nbt[:], mean[:], rstd[:])
                    nb_t = statp.tile([128, 4], F32, tag="nb")
                    nc.vector.tensor_scalar_mul(nb_t[:], nbt[:], -1.0)
                    pend.append((bb, t_sb, rstd, nb_t))

                    # ---- deferred GELU + a2a writes for finished group ----
                    if bb == G - 1:
                        if prev_gelu is not None:
                            for inst in grp_tbl_insts:
                                add_dep_helper(inst.ins, prev_gelu.ins,
                                               sync=False,
                                               reason="act-table grouping")
                        last_tbl = grp_tbl_insts[-1]
                        grp_tbl_insts = []
                        for pb, tl, rl, nl in pend:
                            for st in range(NST):
                                act_sb = actp.tile([128, H], BF16, tag="act")
                                if ln_trivial:
                                    gi = nc.scalar.activation(
                                        act_sb[:], tl[:, st * H:(st + 1) * H],
                                        AF.Gelu,
                                        bias=nl[:, st:st + 1],
                                        scale=rl[:, st:st + 1])
                                else:
                                    nrm = tp.tile([128, H], BF16, tag="nrm")
                                    nc.scalar.activation(
                                        nrm[:], tl[:, st * H:(st + 1) * H],
                                        AF.Identity,
                                        bias=nl[:, st:st + 1],
                                        scale=rl[:, st:st + 1])
                                    nc.vector.tensor_mul(nrm[:], nrm[:],
                                                         lng_sb[:])
                                    nc.vector.tensor_add(nrm[:], nrm[:],
                                                         lnb_sb[:])
                                    gi = nc.scalar.activation(
                                        act_sb[:], nrm[:], AF.Gelu)
                                add_dep_helper(gi.ins, last_tbl.ins,
                                               sync=False,
                                               reason="act-table grouping")
                                nc.scalar.dma_start(a2a_in[ch][2 * st, pb],
                                                    act_sb[0:64, :])
                                nc.scalar.dma_start(a2a_in[ch][2 * st + 1, pb],
                                                    act_sb[64:128, :])
                                prev_gelu = gi
                        pend = []
                        # fire this chunk's AllToAll (overlaps with compute)
                        nc.gpsimd.collective_compute(
                            "AllToAll", ALU.bypass, replica_groups=rg,
                            ins=[a2a_in[ch].opt()],
                            outs=[a2a_out[ch * 64:(ch + 1) * 64, :].opt()])

            # ---- phase 8: y_part[r, d] = sum_sh actT[sh, r] * owT[sh, d] ----
            # a2a_out rows are RS-order batches; XBAR transpose-DMA delivers
            # [128 sh, 256 r] stationary tiles directly.
            with (
                tc.tile_pool(name="p8a", bufs=6) as p8ap,
                tc.tile_pool(name="p8w", bufs=6) as p8wp,
                tc.tile_pool(name="ysb", bufs=2) as ysbp,
                tc.tile_pool(name="yps", bufs=2, space="PSUM") as yps,
            ):
                ypsum = []
                for _bt in range(2):
                    yp_t = yps.tile([128, D], F32, tag="yps")
                    ypsum.append(yp_t)
                for c in range(NC_T):
                    at = p8ap.tile([128, 256], BF16, tag="at")
                    nc.scalar.dma_start_transpose(
                        at[:], a2a_out[:, c * 128:(c + 1) * 128])
                    ow_t = p8wp.tile([128, D], BF16, tag="ow")
                    nc.sync.dma_start(ow_t[:], owT[c])
                    for bt in range(2):
                        nc.tensor.matmul(
                            ypsum[bt][:], at[:, bt * 128:(bt + 1) * 128],
                            ow_t[:],
                            start=(c == 0), stop=(c == NC_T - 1))
                for bt in range(2):
                    y_sb = ysbp.tile([128, D], F32, tag="ysb")
                    nc.vector.tensor_add(y_sb[:], ypsum[bt][:], outb_sb[:])
                    nc.sync.dma_start(y_part[bt * 128:(bt + 1) * 128, :],
                                      y_sb[:])

                nc.gpsimd.collective_compute(
                    "ReduceScatter", ALU.add, replica_groups=rg,
                    ins=[y_part.opt()], outs=[y_red.opt()])
                nc.sync.dma_start(y_out[:], y_red[:])

    nc.compile()
    return nc


_CACHE = {}


def _get_program(ln_trivial):
    if ln_trivial not in _CACHE:
        _CACHE[ln_trivial] = _build(ln_trivial)
    return _CACHE[ln_trivial]


def _prep_inputs(x, k_w, k_b, q_w, q_b, v_w, v_b, attn_bias, ln_g, ln_b,
                 out_w, out_b):
    ln_trivial = bool(np.all(ln_g == 1.0) and np.all(ln_b == 0.0))
    kq_wT = np.ascontiguousarray(
        np.concatenate([k_w.T, q_w.T], axis=1)).astype(BFNP).reshape(
            NDT, 128, 2 * H)
    v_wT = np.ascontiguousarray(v_w.T).astype(BFNP).reshape(NDT, 128, H)
    kqb_row = np.concatenate([k_b, q_b])[None, :].astype(BFNP)
    v_b2 = np.ascontiguousarray(v_b.reshape(NHT, 128, 1)).astype(np.float32)
    ab = np.ascontiguousarray(
        attn_bias.reshape(NST, 128, H).transpose(1, 0, 2).reshape(
            128, NST * H)).astype(BFNP)
    outb8 = np.ascontiguousarray(
        np.tile((out_b / N_CORES)[None, :], (128, 1))).astype(np.float32)
    owT_full = np.ascontiguousarray(out_w.T)  # [S*H, D]
    shared = dict(kq_wT=kq_wT, v_wT=v_wT, kqb_row=kqb_row, v_b2=v_b2, ab=ab,
                  outb8=outb8, ones128=np.ones((128, 128), BFNP))
    if not ln_trivial:
        shared["lng"] = np.tile(ln_g[None, :], (128, 1)).astype(BFNP)
        shared["lnb"] = np.tile(ln_b[None, :], (128, 1)).astype(BFNP)
    in_maps = []
    for i in range(N_CORES):
        xi = np.ascontiguousarray(
            x[i * NB:(i + 1) * NB].transpose(0, 2, 1)).astype(BFNP).reshape(
                NB, NDT, 128, S)
        owi = np.ascontiguousarray(
            owT_full[i * SLICE:(i + 1) * SLICE]).astype(BFNP).reshape(
                NC_T, 128, D)
        m = dict(shared)
        m["xT"] = xi
        m["owT"] = owi
        in_maps.append(m)
    return ln_trivial, in_maps


def _row_perm():
    """RS-row r -> global batch index. Row order is (chunk, src, bb)."""
    r = np.arange(N_CORES * NB)
    chk = r // (N_CORES * G)
    src = (r % (N_CORES * G)) // G
    bb = r % G
    return src * NB + chk * G + bb


def _gather(res):
    shards = [np.asarray(res.results[i]["y"], dtype=np.float32)
              for i in range(N_CORES)]
    ycat = np.concatenate(shards, axis=0)  # [256, 512] in RS row order
    y_full = np.empty_like(ycat)
    y_full[_row_perm()] = ycat
    return y_full


def kernel(**inputs):
    xs = {k: np.asarray(v, dtype=np.float32) for k, v in inputs.items()}
    ln_trivial, in_maps = _prep_inputs(
        xs["x"], xs["k_w"], xs["k_b"], xs["q_w"], xs["q_b"], xs["v_w"],
        xs["v_b"], xs["attn_bias"], xs["ln_g"], xs["ln_b"], xs["out_w"],
        xs["out_b"])
    nc = _get_program(ln_trivial)
    res = run_bass_kernel_spmd(nc, in_maps, core_ids=list(range(N_CORES)))
    return _gather(res).reshape(B, 1, D).astype(np.float32)


# revision 13
# speedup vs baseline: 1.6522x; 1.6522x over previous
"""Trainium2 Bass kernel for the fused attention+LN+GELU+projection module.

Shapes (hardcoded): x [B=256, S=512, D=512]; k/q/v_w [H=256, D]; attn_bias [S, H];
out_w [D, S*H]; output [B, 1, D].

Distribution across 8 NeuronCores (v2, bf16):
 - phases 1-7 (QKV proj, scores, softmax, apply, +bias, LN, GELU): data-parallel
   over batch, 32 batches/core, all compute in bf16 (fp32 PSUM accumulate).
 - activations stream out in 4 AllToAll chunks (8 batches/group) overlapped
   with attention compute; phase 8 (y = act @ out_w.T) reads the redistributed
   activations via XBAR transpose-DMA (no PE transposes), accumulates partial
   products over this core's 1/8 slice of the contraction dim, and a
   ReduceScatter leaves each core with its 32-row shard of the output.
"""

import sys

sys.path.insert(0, "/opt/trn_rl_repo")

import ml_dtypes
import numpy as np

import concourse.bacc as bacc
import concourse.tile as tile
from concourse import mybir
from concourse.bass_utils import run_bass_kernel_spmd
from concourse.hw_specs import get_activation_tables
from concourse.tile_rust import add_dep_helper
import bass_rust as _bass_rust

N_CORES = 8
B, S, H, D = 256, 512, 256, 512
NB = B // N_CORES          # batches per core
SCALE = 1.0 / (B ** 0.5)   # score scale (batch-size based, faithful to ref)
LN_EPS = 1e-5
NDT = D // 128             # 4 d-tiles
NST = S // 128             # 4 s-tiles
NHT = H // 128             # 2 h-tiles
SLICE = (S // N_CORES) * H  # 16384 contraction elems per core
NC_T = SLICE // 128        # 128 contraction tiles per core
G = 8                      # batches per ACT-table/A2A group
NCH = NB // G              # 4 A2A chunks

BF16 = mybir.dt.bfloat16
F32 = mybir.dt.float32
AF = mybir.ActivationFunctionType
ALU = mybir.AluOpType
BFNP = ml_dtypes.bfloat16


class _Bacc(bacc.Bacc):
    """Bacc whose activation-table binding is restricted so that exp/ln are
    only servable by natural_log_exp_and_others and gelu by gelu_and_others.
    Avoids per-op ACT_TABLE_LOAD thrash (~2.7us each) from the default
    first-match binding."""

    def insert_act_table_loads(self):
        has_activation = any(
            isinstance(i, mybir.InstActivation)
            for b in self.main_func.blocks
            for i in b.instructions
        )
        if not has_activation:
            return
        keep = {"natural_log_exp_and_others", "gelu_and_others"}
        strip = {AF.Exp, AF.Ln, AF.Gelu}
        tables = []
        for name, funcs in get_activation_tables(self.m.arch).items():
            if name not in keep:
                funcs = funcs - strip
            tables.append((name, funcs))
        _bass_rust.insert_act_table_loads(self, tables)


def _build(ln_trivial: bool):
    nc = _Bacc("TRN2", target_bir_lowering=False, debug=False,
               num_devices=N_CORES)

    # ---- DRAM I/O ----
    xT = nc.dram_tensor("xT", [NB, NDT, 128, S], BF16, kind="ExternalInput").ap()
    kq_wT = nc.dram_tensor("kq_wT", [NDT, 128, 2 * H], BF16, kind="ExternalInput").ap()
    v_wT = nc.dram_tensor("v_wT", [NDT, 128, H], BF16, kind="ExternalInput").ap()
    kqbb = nc.dram_tensor("kqbb", [128, 2 * 2 * H], BF16, kind="ExternalInput").ap()
    v_b2 = nc.dram_tensor("v_b2", [NHT, 128, 1], F32, kind="ExternalInput").ap()
    ab = nc.dram_tensor("ab", [128, NST * H], BF16, kind="ExternalInput").ap()
    outb8 = nc.dram_tensor("outb8", [128, D], F32, kind="ExternalInput").ap()
    ones128 = nc.dram_tensor("ones128", [128, 128], BF16, kind="ExternalInput").ap()
    owT = nc.dram_tensor("owT", [NC_T // 4, 128, 4 * D], BF16, kind="ExternalInput").ap()
    if not ln_trivial:
        lng = nc.dram_tensor("lng", [128, H], BF16, kind="ExternalInput").ap()
        lnb = nc.dram_tensor("lnb", [128, H], BF16, kind="ExternalInput").ap()
    y_out = nc.dram_tensor("y", [NB, D], F32, kind="ExternalOutput").ap()

    # internal DRAM (collective buffers)
    a2a_in = [
        nc.dram_tensor(f"a2a_in{g}", [N_CORES, G, S // N_CORES, H], BF16).ap()
        for g in range(NCH)
    ]
    a2a_out = nc.dram_tensor("a2a_out", [N_CORES * NB, SLICE], BF16).ap()
    y_part = nc.dram_tensor("y_part", [N_CORES * NB, D], F32).ap()
    y_red = nc.dram_tensor("y_red", [NB, D], F32).ap()

    rg = [list(range(N_CORES))]

    with tile.TileContext(nc) as tc:
        with (
            tc.tile_pool(name="const", bufs=1) as constp,
            tc.tile_pool(name="xt", bufs=12) as xtp,
            tc.tile_pool(name="kqsb", bufs=3) as kqp,
            tc.tile_pool(name="vtsb", bufs=3) as vtp,
            tc.tile_pool(name="esb", bufs=3) as ep,
            tc.tile_pool(name="wsb", bufs=3) as wp,
            tc.tile_pool(name="sden", bufs=3) as sdp,
            tc.tile_pool(name="tsb", bufs=G + 3) as tp,
            tc.tile_pool(name="sqsb", bufs=3) as sqp,
            tc.tile_pool(name="actsb", bufs=8) as actp,
            tc.tile_pool(name="stat", bufs=4 * G + 8) as statp,
            tc.tile_pool(name="tmp4", bufs=12) as tmpp,
        ):
            # ---- persistent constants ----
            kqw_sb = []
            vw_sb = []
            for dt_ in range(NDT):
                t = constp.tile([128, 2 * H], BF16, tag=f"kqw{dt_}")
                nc.sync.dma_start(t[:], kq_wT[dt_])
                kqw_sb.append(t)
                t = constp.tile([128, H], BF16, tag=f"vw{dt_}")
                nc.sync.dma_start(t[:], v_wT[dt_])
                vw_sb.append(t)
            kqbb_sb = constp.tile([128, 2 * 2 * H], BF16, tag="kqbb")
            nc.sync.dma_start(kqbb_sb[:], kqbb[:])
            vb_sb = []
            for ht in range(NHT):
                t = constp.tile([128, 1], F32, tag=f"vb{ht}")
                nc.sync.dma_start(t[:], v_b2[ht])
                vb_sb.append(t)
            ab_sb = constp.tile([128, NST * H], BF16, tag="ab")
            nc.sync.dma_start(ab_sb[:], ab[:])
            outb_sb = constp.tile([128, D], F32, tag="outb")
            nc.sync.dma_start(outb_sb[:], outb8[:])
            if not ln_trivial:
                lng_sb = constp.tile([128, H], BF16, tag="lng")
                nc.sync.dma_start(lng_sb[:], lng[:])
                lnb_sb = constp.tile([128, H], BF16, tag="lnb")
                nc.sync.dma_start(lnb_sb[:], lnb[:])
            ones_sb = constp.tile([128, 128], BF16, tag="ones")
            nc.sync.dma_start(ones_sb[:], ones128[:])
            ones_col = ones_sb[:, 0:1]   # [128, 1] lhsT for partition sums
            ones_row = ones_sb[0:1, :]   # [1, 128] lhsT for broadcasts
            eps_sb = constp.tile([128, 1], F32, tag="eps")
            nc.gpsimd.memset(eps_sb[:], LN_EPS)

            # ---- per-batch attention pipeline ----
            # PSUM budget (8 banks): vt 2 + kq 2x2 + sc 1 + out5 2 = 8
            with (
                tc.tile_pool(name="vtps", bufs=1, space="PSUM") as vtps,
                tc.tile_pool(name="kqps", bufs=2, space="PSUM") as kqpsp,
                tc.tile_pool(name="scps", bufs=1, space="PSUM") as scp,
                tc.tile_pool(name="o5ps", bufs=1, space="PSUM") as o5p,
            ):
                pend = []            # deferred-GELU state per batch in group
                grp_tbl_insts = []   # this group's exp/ln ACT instructions
                prev_gelu = None     # last gelu instruction of previous group
                for b in range(NB):
                    ch = b // G      # a2a chunk this batch belongs to
                    bb = b % G
                    xt = []
                    for dt_ in range(NDT):
                        t = xtp.tile([128, S], BF16, tag="xt")
                        nc.sync.dma_start(t[:], xT[b, dt_])
                        xt.append(t)

                    # vT[h, s] = sum_d v_wT[d, h] * xT[d, s]  (+v_b per-part)
                    vt_sb = vtp.tile([128, 2 * S], BF16, tag="vt")
                    for ht in range(NHT):
                        vps = vtps.tile([128, S], F32, tag="vtps")
                        for dt_ in range(NDT):
                            nc.tensor.matmul(
                                vps[:],
                                vw_sb[dt_][:, ht * 128:(ht + 1) * 128],
                                xt[dt_][:],
                                start=(dt_ == 0), stop=(dt_ == NDT - 1))
                        nc.scalar.activation(
                            vt_sb[:, ht * S:(ht + 1) * S], vps[:],
                            AF.Identity, bias=vb_sb[ht][:])

                    # kq[s, j] = sum_d x[s, d] * [k_wT | q_wT][d, j] + bias
                    # (bias folded in as a K=1 matmul with a ones column)
                    kq_sb = kqp.tile([128, 4 * 512], BF16, tag="kq")
                    for sp in range(2):
                        kqps = kqpsp.tile([128, 1024], F32, tag="kqps")
                        for half in range(2):
                            st = 2 * sp + half
                            c0 = half * 512
                            for dt_ in range(NDT):
                                nc.tensor.matmul(
                                    kqps[:, c0:c0 + 512],
                                    xt[dt_][:, st * 128:(st + 1) * 128],
                                    kqw_sb[dt_][:],
                                    start=(dt_ == 0), stop=(dt_ == NDT - 1))
                        nc.vector.tensor_add(
                            kq_sb[:, sp * 1024:(sp + 1) * 1024],
                            kqps[:], kqbb_sb[:])

                    # scores[h, g] = sum_s k[s, h] q[s, g]; both ht halves in
                    # one bank; e = exp(scores/16)
                    sc = scp.tile([128, 512], F32, tag="scps")
                    for ht in range(NHT):
                        for st in range(NST):
                            nc.tensor.matmul(
                                sc[:, ht * H:(ht + 1) * H],
                                kq_sb[:, st * 512 + ht * 128:
                                      st * 512 + (ht + 1) * 128],
                                kq_sb[:, st * 512 + H:(st + 1) * 512],
                                start=(st == 0), stop=(st == NST - 1))
                    e_sb = ep.tile([128, 512], BF16, tag="e")
                    ei = nc.scalar.activation(e_sb[:], sc[:], AF.Exp,
                                              scale=SCALE)
                    grp_tbl_insts.append(ei)

                    # softmax denominator: reuse the scores bank.
                    # sm[0, g] = sum_h e[h, g]  (ones-matmuls, accumulated)
                    for ht in range(NHT):
                        nc.tensor.matmul(sc[0:1, 0:H], ones_col,
                                         e_sb[:, ht * H:(ht + 1) * H],
                                         start=(ht == 0), stop=(ht == NHT - 1))
                    # rec = 1/denom = exp(-ln(denom)) on the scalar engine
                    # (both funcs live in the natural_log_exp table set)
                    lnd = sdp.tile([1, H], F32, tag="lnd")
                    ldi = nc.scalar.activation(lnd[:], sc[0:1, 0:H], AF.Ln)
                    grp_tbl_insts.append(ldi)
                    sden = sdp.tile([1, H], BF16, tag="sden")
                    rdi = nc.scalar.activation(sden[:], lnd[:], AF.Exp,
                                               scale=-1.0)
                    grp_tbl_insts.append(rdi)
                    # broadcast reciprocal to all partitions (both halves)
                    for ht in range(NHT):
                        nc.tensor.matmul(sc[:, ht * H:(ht + 1) * H],
                                         ones_row, sden[:],
                                         start=True, stop=True)
                    w_sb = wp.tile([128, 512], BF16, tag="w")
                    nc.vector.tensor_mul(w_sb[:], e_sb[:], sc[:])

                    # out5[s, g] = sum_h vT[h, s] w[h, g]; +attn_bias fused
                    # with the LN mean-sum via tensor_tensor_reduce
                    o5 = o5p.tile([128, 1024], F32, tag="o5ps")
                    for st in range(NST):
                        for ht in range(NHT):
                            nc.tensor.matmul(
                                o5[:, st * H:(st + 1) * H],
                                vt_sb[:, ht * S + st * 128:
                                      ht * S + (st + 1) * 128],
                                w_sb[:, ht * H:(ht + 1) * H],
                                start=(ht == 0), stop=(ht == NHT - 1))
                    t_sb = tp.tile([128, 1024], BF16, tag="t")
                    nc.vector.tensor_add(t_sb[:], o5[:], ab_sb[:])
                    sums = statp.tile([128, 4], F32, tag="sums")
                    nc.vector.tensor_reduce(
                        sums[:], t_sb[:].rearrange("p (g c) -> p g c", g=4),
                        axis=mybir.AxisListType.X, op=ALU.add)
                    sq_sb = sqp.tile([128, 1024], BF16, tag="sq")
                    nc.vector.tensor_mul(sq_sb[:], t_sb[:], t_sb[:])
                    sumsq = statp.tile([128, 4], F32, tag="sumsq")
                    nc.vector.tensor_reduce(
                        sumsq[:], sq_sb[:].rearrange("p (g c) -> p g c", g=4),
                        axis=mybir.AxisListType.X, op=ALU.add)
                    # rstd = (var+eps)^-0.5 = exp(-0.5*ln(var+eps))
                    mean = tmpp.tile([128, 4], F32, tag="mean")
                    nc.vector.tensor_scalar_mul(mean[:], sums[:], 1.0 / H)
                    msq = tmpp.tile([128, 4], F32, tag="msq")
                    nc.vector.tensor_mul(msq[:], mean[:], mean[:])
                    q2 = tmpp.tile([128, 4], F32, tag="q2")
                    nc.vector.tensor_scalar_mul(q2[:], sumsq[:], 1.0 / H)
                    var = tmpp.tile([128, 4], F32, tag="var")
                    nc.vector.tensor_sub(var[:], q2[:], msq[:])
                    lnv = tmpp.tile([128, 4], F32, tag="lnv")
                    li = nc.scalar.activation(lnv[:], var[:], AF.Ln,
                                              bias=eps_sb[:])
                    grp_tbl_insts.append(li)
                    rstd = statp.tile([128, 4], F32, tag="rstd")
                    ri = nc.scalar.activation(rstd[:], lnv[:], AF.Exp,
                                              scale=-0.5)
                    grp_tbl_insts.append(ri)
                    nbt = tmpp.tile([128, 4], F32, tag="nbt")
                    nc.vector.tensor_mul(nbt[:], mean[:], rstd[:])
                    nb_t = statp.tile([128, 4], F32, tag="nb")
                    nc.vector.tensor_scalar_mul(nb_t[:], nbt[:], -1.0)
                    pend.append((bb, t_sb, rstd, nb_t))

                    # ---- deferred GELU + a2a writes for finished group ----
                    if bb == G - 1:
                        if prev_gelu is not None:
                            for inst in grp_tbl_insts:
                                add_dep_helper(inst.ins, prev_gelu.ins,
                                               sync=False,
                                               reason="act-table grouping")
                        last_tbl = grp_tbl_insts[-1]
                        grp_tbl_insts = []
                        for pb, tl, rl, nl in pend:
                            for st in range(NST):
                                act_sb = actp.tile([128, H], BF16, tag="act")
                                if ln_trivial:
                                    gi = nc.scalar.activation(
                                        act_sb[:], tl[:, st * H:(st + 1) * H],
                                        AF.Gelu,
                                        bias=nl[:, st:st + 1],
                                        scale=rl[:, st:st + 1])
                                else:
                                    nrm = tp.tile([128, H], BF16, tag="nrm")
                                    nc.scalar.activation(
                                        nrm[:], tl[:, st * H:(st + 1) * H],
                                        AF.Identity,
                                        bias=nl[:, st:st + 1],
                                        scale=rl[:, st:st + 1])
                                    nc.vector.tensor_mul(nrm[:], nrm[:],
                                                         lng_sb[:])
                                    nc.vector.tensor_add(nrm[:], nrm[:],
                                                         lnb_sb[:])
                                    gi = nc.scalar.activation(
                                        act_sb[:], nrm[:], AF.Gelu)
                                add_dep_helper(gi.ins, last_tbl.ins,
                                               sync=False,
                                               reason="act-table grouping")
                                nc.scalar.dma_start(a2a_in[ch][2 * st, pb],
                                                    act_sb[0:64, :])
                                nc.scalar.dma_start(a2a_in[ch][2 * st + 1, pb],
                                                    act_sb[64:128, :])
                                prev_gelu = gi
                        pend = []
                        # fire this chunk's AllToAll (overlaps with compute)
                        nc.gpsimd.collective_compute(
                            "AllToAll", ALU.bypass, replica_groups=rg,
                            ins=[a2a_in[ch].opt()],
                            outs=[a2a_out[ch * 64:(ch + 1) * 64, :].opt()])

            # ---- phase 8: y_part[r, d] = sum_sh actT[sh, r] * owT[sh, d] ----
            # a2a_out rows are RS-order batches; XBAR transpose-DMA delivers
            # [128 sh, 256 r] stationary tiles directly.
            with (
                tc.tile_pool(name="p8a", bufs=6) as p8ap,
                tc.tile_pool(name="p8w", bufs=6) as p8wp,
                tc.tile_pool(name="ysb", bufs=2) as ysbp,
                tc.tile_pool(name="yps", bufs=2, space="PSUM") as yps,
            ):
                ypsum = []
                for _bt in range(2):
                    yp_t = yps.tile([128, D], F32, tag="yps")
                    ypsum.append(yp_t)
                for tcc in range(NC_T // 8):
                    at = p8ap.tile([128, 8 * 256], BF16, tag="at")
                    nc.scalar.dma_start_transpose(
                        at[:].rearrange("p (j b) -> p j b", j=8),
                        a2a_out[:, tcc * 1024:(tcc + 1) * 1024])
                    for cc in range(2 * tcc, 2 * tcc + 2):
                        ow_t = p8wp.tile([128, 4 * D], BF16, tag="ow")
                        nc.sync.dma_start(ow_t[:], owT[cc])
                        for j in range(4):
                            c = 4 * cc + j
                            jj = c - 8 * tcc
                            for bt in range(2):
                                nc.tensor.matmul(
                                    ypsum[bt][:],
                                    at[:, jj * 256 + bt * 128:
                                       jj * 256 + (bt + 1) * 128],
                                    ow_t[:, j * D:(j + 1) * D],
                                    start=(c == 0), stop=(c == NC_T - 1))
                for bt in range(2):
                    y_sb = ysbp.tile([128, D], F32, tag="ysb")
                    nc.vector.tensor_add(y_sb[:], ypsum[bt][:], outb_sb[:])
                    nc.sync.dma_start(y_part[bt * 128:(bt + 1) * 128, :],
                                      y_sb[:])

                nc.gpsimd.collective_compute(
                    "ReduceScatter", ALU.add, replica_groups=rg,
                    ins=[y_part.opt()], outs=[y_red.opt()])
                nc.sync.dma_start(y_out[:], y_red[:])

    nc.compile()
    return nc


_CACHE = {}


def _get_program(ln_trivial):
    if ln_trivial not in _CACHE:
        _CACHE[ln_trivial] = _build(ln_trivial)
    return _CACHE[ln_trivial]


def _prep_inputs(x, k_w, k_b, q_w, q_b, v_w, v_b, attn_bias, ln_g, ln_b,
                 out_w, out_b):
    ln_trivial = bool(np.all(ln_g == 1.0) and np.all(ln_b == 0.0))
    kq_wT = np.ascontiguousarray(
        np.concatenate([k_w.T, q_w.T], axis=1)).astype(BFNP).reshape(
            NDT, 128, 2 * H)
    v_wT = np.ascontiguousarray(v_w.T).astype(BFNP).reshape(NDT, 128, H)
    kqbb = np.tile(np.concatenate([k_b, q_b])[None, :],
                   (128, 2)).astype(BFNP)
    v_b2 = np.ascontiguousarray(v_b.reshape(NHT, 128, 1)).astype(np.float32)
    ab = np.ascontiguousarray(
        attn_bias.reshape(NST, 128, H).transpose(1, 0, 2).reshape(
            128, NST * H)).astype(BFNP)
    outb8 = np.ascontiguousarray(
        np.tile((out_b / N_CORES)[None, :], (128, 1))).astype(np.float32)
    owT_full = np.ascontiguousarray(out_w.T)  # [S*H, D]
    shared = dict(kq_wT=kq_wT, v_wT=v_wT, kqbb=kqbb, v_b2=v_b2, ab=ab,
                  outb8=outb8, ones128=np.ones((128, 128), BFNP))
    if not ln_trivial:
        shared["lng"] = np.tile(ln_g[None, :], (128, 1)).astype(BFNP)
        shared["lnb"] = np.tile(ln_b[None, :], (128, 1)).astype(BFNP)
    in_maps = []
    for i in range(N_CORES):
        xi = np.ascontiguousarray(
            x[i * NB:(i + 1) * NB].transpose(0, 2, 1)).astype(BFNP).reshape(
                NB, NDT, 128, S)
        owi = np.ascontiguousarray(
            owT_full[i * SLICE:(i + 1) * SLICE]).astype(BFNP).reshape(
                NC_T // 4, 4, 128, D).transpose(0, 2, 1, 3).reshape(
                NC_T // 4, 128, 4 * D)
        m = dict(shared)
        m["xT"] = xi
        m["owT"] = owi
        in_maps.append(m)
    return ln_trivial, in_maps


def _row_perm():
    """RS-row r -> global batch index. Row order is (chunk, src, bb)."""
    r = np.arange(N_CORES * NB)
    chk = r // (N_CORES * G)
    src = (r % (N_CORES * G)) // G
    bb = r % G
    return src * NB + chk * G + bb


def _gather(res):
    shards = [np.asarray(res.results[i]["y"], dtype=np.float32)
              for i in range(N_CORES)]
    ycat = np.concatenate(shards, axis=0)  # [256, 512] in RS row order
    y_full = np.empty_like(ycat)
    y_full[_row_perm()] = ycat
    return y_full


def kernel(**inputs):
    xs = {k: np.asarray(v, dtype=np.float32) for k, v in inputs.items()}
    ln_trivial, in_maps = _prep_inputs(
        xs["x"], xs["k_w"], xs["k_b"], xs["q_w"], xs["q_b"], xs["v_w"],
        xs["v_b"], xs["attn_bias"], xs["ln_g"], xs["ln_b"], xs["out_w"],
        xs["out_b"])
    nc = _get_program(ln_trivial)
    res = run_bass_kernel_spmd(nc, in_maps, core_ids=list(range(N_CORES)))
    return _gather(res).reshape(B, 1, D).astype(np.float32)


# revision 14
# speedup vs baseline: 1.7687x; 1.0705x over previous
"""Trainium2 Bass kernel for the fused attention+LN+GELU+projection module.

Shapes (hardcoded): x [B=256, S=512, D=512]; k/q/v_w [H=256, D]; attn_bias [S, H];
out_w [D, S*H]; output [B, 1, D].

Distribution across 8 NeuronCores (v2, bf16):
 - phases 1-7 (QKV proj, scores, softmax, apply, +bias, LN, GELU): data-parallel
   over batch, 32 batches/core, all compute in bf16 (fp32 PSUM accumulate).
 - activations stream out in 4 AllToAll chunks (8 batches/group) overlapped
   with attention compute; phase 8 (y = act @ out_w.T) reads the redistributed
   activations via XBAR transpose-DMA (no PE transposes), accumulates partial
   products over this core's 1/8 slice of the contraction dim, and a
   ReduceScatter leaves each core with its 32-row shard of the output.
"""

import sys

sys.path.insert(0, "/opt/trn_rl_repo")

import ml_dtypes
import numpy as np

import concourse.bacc as bacc
import concourse.tile as tile
from concourse import mybir
from concourse.bass_utils import run_bass_kernel_spmd
from concourse.hw_specs import get_activation_tables
from concourse.tile_rust import add_dep_helper
import bass_rust as _bass_rust

N_CORES = 8
B, S, H, D = 256, 512, 256, 512
NB = B // N_CORES          # batches per core
SCALE = 1.0 / (B ** 0.5)   # score scale (batch-size based, faithful to ref)
LN_EPS = 1e-5
NDT = D // 128             # 4 d-tiles
NST = S // 128             # 4 s-tiles
NHT = H // 128             # 2 h-tiles
SLICE = (S // N_CORES) * H  # 16384 contraction elems per core
NC_T = SLICE // 128        # 128 contraction tiles per core
G = 8                      # batches per ACT-table/A2A group
NCH = NB // G              # 4 A2A chunks

BF16 = mybir.dt.bfloat16
F32 = mybir.dt.float32
AF = mybir.ActivationFunctionType
ALU = mybir.AluOpType
BFNP = ml_dtypes.bfloat16


class _Bacc(bacc.Bacc):
    """Bacc whose activation-table binding is restricted so that exp/ln are
    only servable by natural_log_exp_and_others and gelu by gelu_and_others.
    Avoids per-op ACT_TABLE_LOAD thrash (~2.7us each) from the default
    first-match binding."""

    def insert_act_table_loads(self):
        has_activation = any(
            isinstance(i, mybir.InstActivation)
            for b in self.main_func.blocks
            for i in b.instructions
        )
        if not has_activation:
            return
        keep = {"natural_log_exp_and_others", "gelu_and_others"}
        strip = {AF.Exp, AF.Ln, AF.Gelu}
        tables = []
        for name, funcs in get_activation_tables(self.m.arch).items():
            if name not in keep:
                funcs = funcs - strip
            tables.append((name, funcs))
        _bass_rust.insert_act_table_loads(self, tables)


def _build(ln_trivial: bool):
    nc = _Bacc("TRN2", target_bir_lowering=False, debug=False,
               num_devices=N_CORES)

    # ---- DRAM I/O ----
    xT = nc.dram_tensor("xT", [NB, NDT, 128, S], BF16, kind="ExternalInput").ap()
    kq_wT = nc.dram_tensor("kq_wT", [NDT, 128, 2 * H], BF16, kind="ExternalInput").ap()
    v_wT = nc.dram_tensor("v_wT", [NDT, 128, H], BF16, kind="ExternalInput").ap()
    kqbb = nc.dram_tensor("kqbb", [128, 2 * 2 * H], BF16, kind="ExternalInput").ap()
    v_b2 = nc.dram_tensor("v_b2", [NHT, 128, 1], F32, kind="ExternalInput").ap()
    ab = nc.dram_tensor("ab", [128, NST * H], BF16, kind="ExternalInput").ap()
    outb8 = nc.dram_tensor("outb8", [128, D], F32, kind="ExternalInput").ap()
    ones128 = nc.dram_tensor("ones128", [128, 128], BF16, kind="ExternalInput").ap()
    owT = nc.dram_tensor("owT", [NC_T // 4, 128, 4 * D], BF16, kind="ExternalInput").ap()
    if not ln_trivial:
        lng = nc.dram_tensor("lng", [128, H], BF16, kind="ExternalInput").ap()
        lnb = nc.dram_tensor("lnb", [128, H], BF16, kind="ExternalInput").ap()
    y_out = nc.dram_tensor("y", [NB, D], F32, kind="ExternalOutput").ap()

    # internal DRAM (collective buffers)
    a2a_in = [
        nc.dram_tensor(f"a2a_in{g}", [N_CORES, G, S // N_CORES, H], BF16).ap()
        for g in range(NCH)
    ]
    a2a_out = nc.dram_tensor("a2a_out", [N_CORES * NB, SLICE], BF16).ap()
    y_part = nc.dram_tensor("y_part", [N_CORES * NB, D], F32).ap()
    y_red = nc.dram_tensor("y_red", [NB, D], F32).ap()

    rg = [list(range(N_CORES))]

    with tile.TileContext(nc) as tc:
        with (
            tc.tile_pool(name="const", bufs=1) as constp,
            tc.tile_pool(name="xt", bufs=28) as xtp,
            tc.tile_pool(name="kqsb", bufs=3) as kqp,
            tc.tile_pool(name="vtsb", bufs=3) as vtp,
            tc.tile_pool(name="esb", bufs=3) as ep,
            tc.tile_pool(name="wsb", bufs=3) as wp,
            tc.tile_pool(name="sden", bufs=3) as sdp,
            tc.tile_pool(name="tsb", bufs=G + 3) as tp,
            tc.tile_pool(name="sqsb", bufs=3) as sqp,
            tc.tile_pool(name="actsb", bufs=8) as actp,
            tc.tile_pool(name="stat", bufs=4 * G + 8) as statp,
            tc.tile_pool(name="tmp4", bufs=12) as tmpp,
        ):
            # ---- persistent constants ----
            kqw_sb = []
            vw_sb = []
            for dt_ in range(NDT):
                t = constp.tile([128, 2 * H], BF16, tag=f"kqw{dt_}")
                nc.sync.dma_start(t[:], kq_wT[dt_])
                kqw_sb.append(t)
                t = constp.tile([128, H], BF16, tag=f"vw{dt_}")
                nc.sync.dma_start(t[:], v_wT[dt_])
                vw_sb.append(t)
            kqbb_sb = constp.tile([128, 2 * 2 * H], BF16, tag="kqbb")
            nc.sync.dma_start(kqbb_sb[:], kqbb[:])
            vb_sb = []
            for ht in range(NHT):
                t = constp.tile([128, 1], F32, tag=f"vb{ht}")
                nc.sync.dma_start(t[:], v_b2[ht])
                vb_sb.append(t)
            ab_sb = constp.tile([128, NST * H], BF16, tag="ab")
            nc.sync.dma_start(ab_sb[:], ab[:])
            outb_sb = constp.tile([128, D], F32, tag="outb")
            nc.sync.dma_start(outb_sb[:], outb8[:])
            if not ln_trivial:
                lng_sb = constp.tile([128, H], BF16, tag="lng")
                nc.sync.dma_start(lng_sb[:], lng[:])
                lnb_sb = constp.tile([128, H], BF16, tag="lnb")
                nc.sync.dma_start(lnb_sb[:], lnb[:])
            ones_sb = constp.tile([128, 128], BF16, tag="ones")
            nc.sync.dma_start(ones_sb[:], ones128[:])
            ones_col = ones_sb[:, 0:1]   # [128, 1] lhsT for partition sums
            ones_row = ones_sb[0:1, :]   # [1, 128] lhsT for broadcasts
            eps_sb = constp.tile([128, 1], F32, tag="eps")
            nc.gpsimd.memset(eps_sb[:], LN_EPS)

            # ---- per-batch attention pipeline ----
            # PSUM budget (8 banks): vt 2 + kq 2x2 + sc 1 + out5 2 = 8
            with (
                tc.tile_pool(name="vtps", bufs=1, space="PSUM") as vtps,
                tc.tile_pool(name="kqps", bufs=2, space="PSUM") as kqpsp,
                tc.tile_pool(name="scps", bufs=1, space="PSUM") as scp,
                tc.tile_pool(name="o5ps", bufs=1, space="PSUM") as o5p,
            ):
                pend = []            # deferred-GELU state per batch in group
                grp_tbl_insts = []   # this group's exp/ln ACT instructions
                prev_gelu = None     # last gelu instruction of previous group
                for b in range(NB):
                    ch = b // G      # a2a chunk this batch belongs to
                    bb = b % G
                    xt = []
                    for dt_ in range(NDT):
                        t = xtp.tile([128, S], BF16, tag="xt")
                        nc.sync.dma_start(t[:], xT[b, dt_])
                        xt.append(t)

                    # vT[h, s] = sum_d v_wT[d, h] * xT[d, s]  (+v_b per-part)
                    vt_sb = vtp.tile([128, 2 * S], BF16, tag="vt")
                    for ht in range(NHT):
                        vps = vtps.tile([128, S], F32, tag="vtps")
                        for dt_ in range(NDT):
                            nc.tensor.matmul(
                                vps[:],
                                vw_sb[dt_][:, ht * 128:(ht + 1) * 128],
                                xt[dt_][:],
                                start=(dt_ == 0), stop=(dt_ == NDT - 1))
                        nc.scalar.activation(
                            vt_sb[:, ht * S:(ht + 1) * S], vps[:],
                            AF.Identity, bias=vb_sb[ht][:])

                    # kq[s, j] = sum_d x[s, d] * [k_wT | q_wT][d, j] + bias
                    # (bias folded in as a K=1 matmul with a ones column)
                    kq_sb = kqp.tile([128, 4 * 512], BF16, tag="kq")
                    for sp in range(2):
                        kqps = kqpsp.tile([128, 1024], F32, tag="kqps")
                        for half in range(2):
                            st = 2 * sp + half
                            c0 = half * 512
                            for dt_ in range(NDT):
                                nc.tensor.matmul(
                                    kqps[:, c0:c0 + 512],
                                    xt[dt_][:, st * 128:(st + 1) * 128],
                                    kqw_sb[dt_][:],
                                    start=(dt_ == 0), stop=(dt_ == NDT - 1))
                        nc.vector.tensor_add(
                            kq_sb[:, sp * 1024:(sp + 1) * 1024],
                            kqps[:], kqbb_sb[:])

                    # scores[h, g] = sum_s k[s, h] q[s, g]; both ht halves in
                    # one bank; e = exp(scores/16)
                    sc = scp.tile([128, 512], F32, tag="scps")
                    for ht in range(NHT):
                        for st in range(NST):
                            nc.tensor.matmul(
                                sc[:, ht * H:(ht + 1) * H],
                                kq_sb[:, st * 512 + ht * 128:
                                      st * 512 + (ht + 1) * 128],
                                kq_sb[:, st * 512 + H:(st + 1) * 512],
                                start=(st == 0), stop=(st == NST - 1))
                    e_sb = ep.tile([128, 512], BF16, tag="e")
                    ei = nc.scalar.activation(e_sb[:], sc[:], AF.Exp,
                                              scale=SCALE)
                    grp_tbl_insts.append(ei)

                    # softmax denominator: reuse the scores bank.
                    # sm[0, g] = sum_h e[h, g]  (ones-matmuls, accumulated)
                    for ht in range(NHT):
                        nc.tensor.matmul(sc[0:1, 0:H], ones_col,
                                         e_sb[:, ht * H:(ht + 1) * H],
                                         start=(ht == 0), stop=(ht == NHT - 1))
                    # rec = 1/denom = exp(-ln(denom)) on the scalar engine
                    # (both funcs live in the natural_log_exp table set)
                    lnd = sdp.tile([1, H], F32, tag="lnd")
                    ldi = nc.scalar.activation(lnd[:], sc[0:1, 0:H], AF.Ln)
                    grp_tbl_insts.append(ldi)
                    sden = sdp.tile([1, H], BF16, tag="sden")
                    rdi = nc.scalar.activation(sden[:], lnd[:], AF.Exp,
                                               scale=-1.0)
                    grp_tbl_insts.append(rdi)
                    # broadcast reciprocal to all partitions (both halves)
                    for ht in range(NHT):
                        nc.tensor.matmul(sc[:, ht * H:(ht + 1) * H],
                                         ones_row, sden[:],
                                         start=True, stop=True)
                    w_sb = wp.tile([128, 512], BF16, tag="w")
                    nc.vector.tensor_mul(w_sb[:], e_sb[:], sc[:])

                    # out5[s, g] = sum_h vT[h, s] w[h, g]; +attn_bias fused
                    # with the LN mean-sum via tensor_tensor_reduce
                    o5 = o5p.tile([128, 1024], F32, tag="o5ps")
                    for st in range(NST):
                        for ht in range(NHT):
                            nc.tensor.matmul(
                                o5[:, st * H:(st + 1) * H],
                                vt_sb[:, ht * S + st * 128:
                                      ht * S + (st + 1) * 128],
                                w_sb[:, ht * H:(ht + 1) * H],
                                start=(ht == 0), stop=(ht == NHT - 1))
                    t_sb = tp.tile([128, 1024], BF16, tag="t")
                    nc.vector.tensor_add(t_sb[:], o5[:], ab_sb[:])
                    sums = statp.tile([128, 4], F32, tag="sums")
                    nc.vector.tensor_reduce(
                        sums[:], t_sb[:].rearrange("p (g c) -> p g c", g=4),
                        axis=mybir.AxisListType.X, op=ALU.add)
                    sq_sb = sqp.tile([128, 1024], BF16, tag="sq")
                    nc.vector.tensor_mul(sq_sb[:], t_sb[:], t_sb[:])
                    sumsq = statp.tile([128, 4], F32, tag="sumsq")
                    nc.vector.tensor_reduce(
                        sumsq[:], sq_sb[:].rearrange("p (g c) -> p g c", g=4),
                        axis=mybir.AxisListType.X, op=ALU.add)
                    # rstd = (var+eps)^-0.5 = exp(-0.5*ln(var+eps))
                    mean = tmpp.tile([128, 4], F32, tag="mean")
                    nc.vector.tensor_scalar_mul(mean[:], sums[:], 1.0 / H)
                    msq = tmpp.tile([128, 4], F32, tag="msq")
                    nc.vector.tensor_mul(msq[:], mean[:], mean[:])
                    q2 = tmpp.tile([128, 4], F32, tag="q2")
                    nc.vector.tensor_scalar_mul(q2[:], sumsq[:], 1.0 / H)
                    var = tmpp.tile([128, 4], F32, tag="var")
                    nc.vector.tensor_sub(var[:], q2[:], msq[:])
                    lnv = tmpp.tile([128, 4], F32, tag="lnv")
                    li = nc.scalar.activation(lnv[:], var[:], AF.Ln,
                                              bias=eps_sb[:])
                    grp_tbl_insts.append(li)
                    rstd = statp.tile([128, 4], F32, tag="rstd")
                    ri = nc.scalar.activation(rstd[:], lnv[:], AF.Exp,
                                              scale=-0.5)
                    grp_tbl_insts.append(ri)
                    nbt = tmpp.tile([128, 4], F32, tag="nbt")
                    nc.vector.tensor_mul(nbt[:], mean[:], rstd[:])
                    nb_t = statp.tile([128, 4], F32, tag="nb")
                    nc.vector.tensor_scalar_mul(nb_t[:], nbt[:], -1.0)
                    pend.append((bb, t_sb, rstd, nb_t))

                    # ---- deferred GELU + a2a writes for finished group ----
                    if bb == G - 1:
                        if prev_gelu is not None:
                            for inst in grp_tbl_insts:
                                add_dep_helper(inst.ins, prev_gelu.ins,
                                               sync=False,
                                               reason="act-table grouping")
                        last_tbl = grp_tbl_insts[-1]
                        grp_tbl_insts = []
                        for pb, tl, rl, nl in pend:
                            for st in range(NST):
                                act_sb = actp.tile([128, H], BF16, tag="act")
                                if ln_trivial:
                                    gi = nc.scalar.activation(
                                        act_sb[:], tl[:, st * H:(st + 1) * H],
                                        AF.Gelu,
                                        bias=nl[:, st:st + 1],
                                        scale=rl[:, st:st + 1])
                                else:
                                    nrm = tp.tile([128, H], BF16, tag="nrm")
                                    nc.scalar.activation(
                                        nrm[:], tl[:, st * H:(st + 1) * H],
                                        AF.Identity,
                                        bias=nl[:, st:st + 1],
                                        scale=rl[:, st:st + 1])
                                    nc.vector.tensor_mul(nrm[:], nrm[:],
                                                         lng_sb[:])
                                    nc.vector.tensor_add(nrm[:], nrm[:],
                                                         lnb_sb[:])
                                    gi = nc.scalar.activation(
                                        act_sb[:], nrm[:], AF.Gelu)
                                add_dep_helper(gi.ins, last_tbl.ins,
                                               sync=False,
                                               reason="act-table grouping")
                                nc.sync.dma_start(a2a_in[ch][2 * st, pb],
                                                   act_sb[0:64, :])
                                nc.sync.dma_start(a2a_in[ch][2 * st + 1, pb],
                                                  act_sb[64:128, :])
                                prev_gelu = gi
                        pend = []
                        # fire this chunk's AllToAll (overlaps with compute)
                        nc.gpsimd.collective_compute(
                            "AllToAll", ALU.bypass, replica_groups=rg,
                            ins=[a2a_in[ch].opt()],
                            outs=[a2a_out[ch * 64:(ch + 1) * 64, :].opt()])

            # ---- phase 8: y_part[r, d] = sum_sh actT[sh, r] * owT[sh, d] ----
            # a2a_out rows are RS-order batches; XBAR transpose-DMA delivers
            # [128 sh, 256 r] stationary tiles directly.
            with (
                tc.tile_pool(name="p8a", bufs=6) as p8ap,
                tc.tile_pool(name="p8w", bufs=6) as p8wp,
                tc.tile_pool(name="ysb", bufs=2) as ysbp,
                tc.tile_pool(name="yps", bufs=2, space="PSUM") as yps,
            ):
                ypsum = []
                for _bt in range(2):
                    yp_t = yps.tile([128, D], F32, tag="yps")
                    ypsum.append(yp_t)
                for tcc in range(NC_T // 8):
                    at = p8ap.tile([128, 8 * 256], BF16, tag="at")
                    nc.scalar.dma_start_transpose(
                        at[:].rearrange("p (j b) -> p j b", j=8),
                        a2a_out[:, tcc * 1024:(tcc + 1) * 1024])
                    for cc in range(2 * tcc, 2 * tcc + 2):
                        ow_t = p8wp.tile([128, 4 * D], BF16, tag="ow")
                        nc.sync.dma_start(ow_t[:], owT[cc])
                        for j in range(4):
                            c = 4 * cc + j
                            jj = c - 8 * tcc
                            for bt in range(2):
                                nc.tensor.matmul(
                                    ypsum[bt][:],
                                    at[:, jj * 256 + bt * 128:
                                       jj * 256 + (bt + 1) * 128],
                                    ow_t[:, j * D:(j + 1) * D],
                                    start=(c == 0), stop=(c == NC_T - 1))
                for bt in range(2):
                    y_sb = ysbp.tile([128, D], F32, tag="ysb")
                    nc.vector.tensor_add(y_sb[:], ypsum[bt][:], outb_sb[:])
                    nc.sync.dma_start(y_part[bt * 128:(bt + 1) * 128, :],
                                      y_sb[:])

                nc.gpsimd.collective_compute(
                    "ReduceScatter", ALU.add, replica_groups=rg,
                    ins=[y_part.opt()], outs=[y_red.opt()])
                nc.sync.dma_start(y_out[:], y_red[:])

    nc.compile()
    return nc


_CACHE = {}


def _get_program(ln_trivial):
    if ln_trivial not in _CACHE:
        _CACHE[ln_trivial] = _build(ln_trivial)
    return _CACHE[ln_trivial]


def _prep_inputs(x, k_w, k_b, q_w, q_b, v_w, v_b, attn_bias, ln_g, ln_b,
                 out_w, out_b):
    ln_trivial = bool(np.all(ln_g == 1.0) and np.all(ln_b == 0.0))
    kq_wT = np.ascontiguousarray(
        np.concatenate([k_w.T, q_w.T], axis=1)).astype(BFNP).reshape(
            NDT, 128, 2 * H)
    v_wT = np.ascontiguousarray(v_w.T).astype(BFNP).reshape(NDT, 128, H)
    kqbb = np.tile(np.concatenate([k_b, q_b])[None, :],
                   (128, 2)).astype(BFNP)
    v_b2 = np.ascontiguousarray(v_b.reshape(NHT, 128, 1)).astype(np.float32)
    ab = np.ascontiguousarray(
        attn_bias.reshape(NST, 128, H).transpose(1, 0, 2).reshape(
            128, NST * H)).astype(BFNP)
    outb8 = np.ascontiguousarray(
        np.tile((out_b / N_CORES)[None, :], (128, 1))).astype(np.float32)
    owT_full = np.ascontiguousarray(out_w.T)  # [S*H, D]
    shared = dict(kq_wT=kq_wT, v_wT=v_wT, kqbb=kqbb, v_b2=v_b2, ab=ab,
                  outb8=outb8, ones128=np.ones((128, 128), BFNP))
    if not ln_trivial:
        shared["lng"] = np.tile(ln_g[None, :], (128, 1)).astype(BFNP)
        shared["lnb"] = np.tile(ln_b[None, :], (128, 1)).astype(BFNP)
    in_maps = []
    for i in range(N_CORES):
        xi = np.ascontiguousarray(
            x[i * NB:(i + 1) * NB].transpose(0, 2, 1)).astype(BFNP).reshape(
                NB, NDT, 128, S)
        owi = np.ascontiguousarray(
            owT_full[i * SLICE:(i + 1) * SLICE]).astype(BFNP).reshape(
                NC_T // 4, 4, 128, D).transpose(0, 2, 1, 3).reshape(
                NC_T // 4, 128, 4 * D)
        m = dict(shared)
        m["xT"] = xi
        m["owT"] = owi
        in_maps.append(m)
    return ln_trivial, in_maps


def _row_perm():
    """RS-row r -> global batch index. Row order is (chunk, src, bb)."""
    r = np.arange(N_CORES * NB)
    chk = r // (N_CORES * G)
    src = (r % (N_CORES * G)) // G
    bb = r % G
    return src * NB + chk * G + bb


def _gather(res):
    shards = [np.asarray(res.results[i]["y"], dtype=np.float32)
              for i in range(N_CORES)]
    ycat = np.concatenate(shards, axis=0)  # [256, 512] in RS row order
    y_full = np.empty_like(ycat)
    y_full[_row_perm()] = ycat
    return y_full


def kernel(**inputs):
    xs = {k: np.asarray(v, dtype=np.float32) for k, v in inputs.items()}
    ln_trivial, in_maps = _prep_inputs(
        xs["x"], xs["k_w"], xs["k_b"], xs["q_w"], xs["q_b"], xs["v_w"],
        xs["v_b"], xs["attn_bias"], xs["ln_g"], xs["ln_b"], xs["out_w"],
        xs["out_b"])
    nc = _get_program(ln_trivial)
    res = run_bass_kernel_spmd(nc, in_maps, core_ids=list(range(N_CORES)))
    return _gather(res).reshape(B, 1, D).astype(np.float32)


# revision 15
# speedup vs baseline: 1.9850x; 1.1223x over previous
"""Trainium2 Bass kernel for the fused attention+LN+GELU+projection module.

Shapes (hardcoded): x [B=256, S=512, D=512]; k/q/v_w [H=256, D]; attn_bias [S, H];
out_w [D, S*H]; output [B, 1, D].

Distribution across 8 NeuronCores (v2, bf16):
 - phases 1-7 (QKV proj, scores, softmax, apply, +bias, LN, GELU): data-parallel
   over batch, 32 batches/core, all compute in bf16 (fp32 PSUM accumulate).
 - activations stream out in 4 AllToAll chunks (8 batches/group) overlapped
   with attention compute; phase 8 (y = act @ out_w.T) reads the redistributed
   activations via XBAR transpose-DMA (no PE transposes), accumulates partial
   products over this core's 1/8 slice of the contraction dim, and a
   ReduceScatter leaves each core with its 32-row shard of the output.
"""

import sys

sys.path.insert(0, "/opt/trn_rl_repo")

import ml_dtypes
import numpy as np

import concourse.bacc as bacc
import concourse.tile as tile
from concourse import mybir
from concourse.bass_utils import run_bass_kernel_spmd
from concourse.hw_specs import get_activation_tables
from concourse.tile_rust import add_dep_helper
import bass_rust as _bass_rust

N_CORES = 8
B, S, H, D = 256, 512, 256, 512
NB = B // N_CORES          # batches per core
SCALE = 1.0 / (B ** 0.5)   # score scale (batch-size based, faithful to ref)
LN_EPS = 1e-5
NDT = D // 128             # 4 d-tiles
NST = S // 128             # 4 s-tiles
NHT = H // 128             # 2 h-tiles
SLICE = (S // N_CORES) * H  # 16384 contraction elems per core
NC_T = SLICE // 128        # 128 contraction tiles per core
G = 8                      # batches per ACT-table/A2A group
NCH = NB // G              # 4 A2A chunks

BF16 = mybir.dt.bfloat16
F32 = mybir.dt.float32
AF = mybir.ActivationFunctionType
ALU = mybir.AluOpType
BFNP = ml_dtypes.bfloat16


class _Bacc(bacc.Bacc):
    """Bacc whose activation-table binding is restricted so that exp/ln are
    only servable by natural_log_exp_and_others and gelu by gelu_and_others.
    Avoids per-op ACT_TABLE_LOAD thrash (~2.7us each) from the default
    first-match binding."""

    def insert_act_table_loads(self):
        has_activation = any(
            isinstance(i, mybir.InstActivation)
            for b in self.main_func.blocks
            for i in b.instructions
        )
        if not has_activation:
            return
        keep = {"natural_log_exp_and_others", "gelu_and_others"}
        strip = {AF.Exp, AF.Ln, AF.Gelu}
        tables = []
        for name, funcs in get_activation_tables(self.m.arch).items():
            if name not in keep:
                funcs = funcs - strip
            tables.append((name, funcs))
        _bass_rust.insert_act_table_loads(self, tables)


def _build(ln_trivial: bool):
    nc = _Bacc("TRN2", target_bir_lowering=False, debug=False,
               num_devices=N_CORES)

    # ---- DRAM I/O ----
    xT = nc.dram_tensor("xT", [NB, NDT, 128, S], BF16, kind="ExternalInput").ap()
    kq_wT = nc.dram_tensor("kq_wT", [NDT, 128, 2 * H], BF16, kind="ExternalInput").ap()
    v_wT = nc.dram_tensor("v_wT", [NDT, 128, H], BF16, kind="ExternalInput").ap()
    kqbb = nc.dram_tensor("kqbb", [128, 2 * 2 * H], BF16, kind="ExternalInput").ap()
    v_b2 = nc.dram_tensor("v_b2", [NHT, 128, 1], F32, kind="ExternalInput").ap()
    ab = nc.dram_tensor("ab", [128, NST * H], BF16, kind="ExternalInput").ap()
    outb8 = nc.dram_tensor("outb8", [128, D], F32, kind="ExternalInput").ap()
    ones128 = nc.dram_tensor("ones128", [128, 128], BF16, kind="ExternalInput").ap()
    owT = nc.dram_tensor("owT", [NC_T // 4, 128, 4 * D], BF16, kind="ExternalInput").ap()
    if not ln_trivial:
        lng = nc.dram_tensor("lng", [128, H], BF16, kind="ExternalInput").ap()
        lnb = nc.dram_tensor("lnb", [128, H], BF16, kind="ExternalInput").ap()
    y_out = nc.dram_tensor("y", [NB, D], F32, kind="ExternalOutput").ap()

    # internal DRAM (collective buffers)
    a2a_in = [
        nc.dram_tensor(f"a2a_in{g}", [N_CORES, G, S // N_CORES, H], BF16).ap()
        for g in range(NCH)
    ]
    a2a_out = nc.dram_tensor("a2a_out", [N_CORES * NB, SLICE], BF16).ap()
    y_part = nc.dram_tensor("y_part", [N_CORES * NB, D], F32).ap()
    y_red = nc.dram_tensor("y_red", [NB, D], F32).ap()

    rg = [list(range(N_CORES))]

    with tile.TileContext(nc) as tc:
        with (
            tc.tile_pool(name="const", bufs=1) as constp,
            tc.tile_pool(name="xt", bufs=28) as xtp,
            tc.tile_pool(name="kqsb", bufs=3) as kqp,
            tc.tile_pool(name="vtsb", bufs=3) as vtp,
            tc.tile_pool(name="esb", bufs=3) as ep,
            tc.tile_pool(name="wsb", bufs=3) as wp,
            tc.tile_pool(name="sden", bufs=3) as sdp,
            tc.tile_pool(name="tsb", bufs=G + 3) as tp,
            tc.tile_pool(name="sqsb", bufs=3) as sqp,
            tc.tile_pool(name="actsb", bufs=8) as actp,
            tc.tile_pool(name="stat", bufs=4 * G + 8) as statp,
            tc.tile_pool(name="tmp4", bufs=12) as tmpp,
        ):
            # ---- persistent constants ----
            kqw_sb = []
            vw_sb = []
            for dt_ in range(NDT):
                t = constp.tile([128, 2 * H], BF16, tag=f"kqw{dt_}")
                nc.sync.dma_start(t[:], kq_wT[dt_])
                kqw_sb.append(t)
                t = constp.tile([128, H], BF16, tag=f"vw{dt_}")
                nc.sync.dma_start(t[:], v_wT[dt_])
                vw_sb.append(t)
            kqbb_sb = constp.tile([128, 2 * 2 * H], BF16, tag="kqbb")
            nc.sync.dma_start(kqbb_sb[:], kqbb[:])
            vb_sb = []
            for ht in range(NHT):
                t = constp.tile([128, 1], F32, tag=f"vb{ht}")
                nc.sync.dma_start(t[:], v_b2[ht])
                vb_sb.append(t)
            ab_sb = constp.tile([128, NST * H], BF16, tag="ab")
            nc.sync.dma_start(ab_sb[:], ab[:])
            outb_sb = constp.tile([128, D], F32, tag="outb")
            nc.sync.dma_start(outb_sb[:], outb8[:])
            if not ln_trivial:
                lng_sb = constp.tile([128, H], BF16, tag="lng")
                nc.sync.dma_start(lng_sb[:], lng[:])
                lnb_sb = constp.tile([128, H], BF16, tag="lnb")
                nc.sync.dma_start(lnb_sb[:], lnb[:])
            ones_sb = constp.tile([128, 128], BF16, tag="ones")
            nc.sync.dma_start(ones_sb[:], ones128[:])
            ones_col = ones_sb[:, 0:1]   # [128, 1] lhsT for partition sums
            ones_row = ones_sb[0:1, :]   # [1, 128] lhsT for broadcasts
            eps_sb = constp.tile([128, 1], F32, tag="eps")
            nc.gpsimd.memset(eps_sb[:], LN_EPS)

            # ---- per-batch attention pipeline ----
            # PSUM budget (8 banks): vt 2 + kq 2x2 + sc 1 + out5 2 = 8
            with (
                tc.tile_pool(name="vtps", bufs=1, space="PSUM") as vtps,
                tc.tile_pool(name="kqps", bufs=2, space="PSUM") as kqpsp,
                tc.tile_pool(name="scps", bufs=1, space="PSUM") as scp,
                tc.tile_pool(name="o5ps", bufs=1, space="PSUM") as o5p,
            ):
                pend = []            # deferred-GELU state per batch in group
                grp_tbl_insts = []   # this group's exp/ln ACT instructions
                prev_gelu = None     # last gelu instruction of previous group
                for b in range(NB):
                    ch = b // G      # a2a chunk this batch belongs to
                    bb = b % G
                    xt = []
                    for dt_ in range(NDT):
                        t = xtp.tile([128, S], BF16, tag="xt")
                        nc.sync.dma_start(t[:], xT[b, dt_])
                        xt.append(t)

                    # vT[h, s] = sum_d v_wT[d, h] * xT[d, s]  (+v_b per-part)
                    vt_sb = vtp.tile([128, 2 * S], BF16, tag="vt")
                    for ht in range(NHT):
                        vps = vtps.tile([128, S], F32, tag="vtps")
                        for dt_ in range(NDT):
                            nc.tensor.matmul(
                                vps[:],
                                vw_sb[dt_][:, ht * 128:(ht + 1) * 128],
                                xt[dt_][:],
                                start=(dt_ == 0), stop=(dt_ == NDT - 1))
                        nc.scalar.activation(
                            vt_sb[:, ht * S:(ht + 1) * S], vps[:],
                            AF.Identity, bias=vb_sb[ht][:])

                    # kq[s, j] = sum_d x[s, d] * [k_wT | q_wT][d, j] + bias
                    # (bias folded in as a K=1 matmul with a ones column)
                    kq_sb = kqp.tile([128, 4 * 512], BF16, tag="kq")
                    for sp in range(2):
                        kqps = kqpsp.tile([128, 1024], F32, tag="kqps")
                        for half in range(2):
                            st = 2 * sp + half
                            c0 = half * 512
                            for dt_ in range(NDT):
                                nc.tensor.matmul(
                                    kqps[:, c0:c0 + 512],
                                    xt[dt_][:, st * 128:(st + 1) * 128],
                                    kqw_sb[dt_][:],
                                    start=(dt_ == 0), stop=(dt_ == NDT - 1))
                        nc.vector.tensor_add(
                            kq_sb[:, sp * 1024:(sp + 1) * 1024],
                            kqps[:], kqbb_sb[:])

                    # scores[h, g] = sum_s k[s, h] q[s, g]; both ht halves in
                    # one bank; e = exp(scores/16)
                    sc = scp.tile([128, 512], F32, tag="scps")
                    for ht in range(NHT):
                        for st in range(NST):
                            nc.tensor.matmul(
                                sc[:, ht * H:(ht + 1) * H],
                                kq_sb[:, st * 512 + ht * 128:
                                      st * 512 + (ht + 1) * 128],
                                kq_sb[:, st * 512 + H:(st + 1) * 512],
                                start=(st == 0), stop=(st == NST - 1))
                    e_sb = ep.tile([128, 512], BF16, tag="e")
                    ei = nc.scalar.activation(e_sb[:], sc[:], AF.Exp,
                                              scale=SCALE)
                    grp_tbl_insts.append(ei)

                    # softmax denominator: reuse the scores bank.
                    # sm[0, g] = sum_h e[h, g]  (ones-matmuls, accumulated)
                    for ht in range(NHT):
                        nc.tensor.matmul(sc[0:1, 0:H], ones_col,
                                         e_sb[:, ht * H:(ht + 1) * H],
                                         start=(ht == 0), stop=(ht == NHT - 1))
                    # rec = 1/denom = exp(-ln(denom)) on the scalar engine
                    # (both funcs live in the natural_log_exp table set)
                    lnd = sdp.tile([1, H], F32, tag="lnd")
                    ldi = nc.scalar.activation(lnd[:], sc[0:1, 0:H], AF.Ln)
                    grp_tbl_insts.append(ldi)
                    sden = sdp.tile([1, H], BF16, tag="sden")
                    rdi = nc.scalar.activation(sden[:], lnd[:], AF.Exp,
                                               scale=-1.0)
                    grp_tbl_insts.append(rdi)
                    # broadcast reciprocal to all partitions (both halves)
                    for ht in range(NHT):
                        nc.tensor.matmul(sc[:, ht * H:(ht + 1) * H],
                                         ones_row, sden[:],
                                         start=True, stop=True)
                    w_sb = wp.tile([128, 512], BF16, tag="w")
                    nc.vector.tensor_mul(w_sb[:], e_sb[:], sc[:])

                    # out5[s, g] = sum_h vT[h, s] w[h, g]; +attn_bias fused
                    # with the LN mean-sum via tensor_tensor_reduce
                    o5 = o5p.tile([128, 1024], F32, tag="o5ps")
                    for st in range(NST):
                        for ht in range(NHT):
                            nc.tensor.matmul(
                                o5[:, st * H:(st + 1) * H],
                                vt_sb[:, ht * S + st * 128:
                                      ht * S + (st + 1) * 128],
                                w_sb[:, ht * H:(ht + 1) * H],
                                start=(ht == 0), stop=(ht == NHT - 1))
                    t_sb = tp.tile([128, 1024], BF16, tag="t")
                    nc.vector.tensor_add(t_sb[:], o5[:], ab_sb[:])
                    sums = statp.tile([128, 4], F32, tag="sums")
                    nc.vector.tensor_reduce(
                        sums[:], t_sb[:].rearrange("p (g c) -> p g c", g=4),
                        axis=mybir.AxisListType.X, op=ALU.add)
                    sq_sb = sqp.tile([128, 1024], BF16, tag="sq")
                    nc.vector.tensor_mul(sq_sb[:], t_sb[:], t_sb[:])
                    sumsq = statp.tile([128, 4], F32, tag="sumsq")
                    nc.vector.tensor_reduce(
                        sumsq[:], sq_sb[:].rearrange("p (g c) -> p g c", g=4),
                        axis=mybir.AxisListType.X, op=ALU.add)
                    # rstd = (var+eps)^-0.5 = exp(-0.5*ln(var+eps))
                    mean = tmpp.tile([128, 4], F32, tag="mean")
                    nc.vector.tensor_scalar_mul(mean[:], sums[:], 1.0 / H)
                    msq = tmpp.tile([128, 4], F32, tag="msq")
                    nc.vector.tensor_mul(msq[:], mean[:], mean[:])
                    q2 = tmpp.tile([128, 4], F32, tag="q2")
                    nc.vector.tensor_scalar_mul(q2[:], sumsq[:], 1.0 / H)
                    var = tmpp.tile([128, 4], F32, tag="var")
                    nc.vector.tensor_sub(var[:], q2[:], msq[:])
                    lnv = tmpp.tile([128, 4], F32, tag="lnv")
                    li = nc.scalar.activation(lnv[:], var[:], AF.Ln,
                                              bias=eps_sb[:])
                    grp_tbl_insts.append(li)
                    rstd = statp.tile([128, 4], F32, tag="rstd")
                    ri = nc.scalar.activation(rstd[:], lnv[:], AF.Exp,
                                              scale=-0.5)
                    grp_tbl_insts.append(ri)
                    nbt = tmpp.tile([128, 4], F32, tag="nbt")
                    nc.vector.tensor_mul(nbt[:], mean[:], rstd[:])
                    nb_t = statp.tile([128, 4], F32, tag="nb")
                    nc.vector.tensor_scalar_mul(nb_t[:], nbt[:], -1.0)
                    # normalize on DVE now; only the (table-gated) GELU is
                    # deferred to the group boundary
                    nrm = tp.tile([128, 1024], BF16, tag="nrm")
                    for st in range(NST):
                        nc.vector.tensor_scalar(
                            nrm[:, st * H:(st + 1) * H],
                            t_sb[:, st * H:(st + 1) * H],
                            rstd[:, st:st + 1], nb_t[:, st:st + 1],
                            ALU.mult, ALU.add)
                    pend.append((bb, nrm))

                    # ---- deferred GELU + a2a writes for finished group ----
                    if bb == G - 1:
                        if prev_gelu is not None:
                            for inst in grp_tbl_insts:
                                add_dep_helper(inst.ins, prev_gelu.ins,
                                               sync=False,
                                               reason="act-table grouping")
                        last_tbl = grp_tbl_insts[-1]
                        grp_tbl_insts = []
                        for pb, nl in pend:
                            gin = nl[:]
                            if not ln_trivial:
                                sc2 = tp.tile([128, 1024], BF16, tag="sc2")
                                for st in range(NST):
                                    nc.vector.tensor_mul(
                                        sc2[:, st * H:(st + 1) * H],
                                        nl[:, st * H:(st + 1) * H],
                                        lng_sb[:])
                                    nc.vector.tensor_add(
                                        sc2[:, st * H:(st + 1) * H],
                                        sc2[:, st * H:(st + 1) * H],
                                        lnb_sb[:])
                                gin = sc2[:]
                            act_sb = actp.tile([128, 1024], BF16, tag="act")
                            gi = nc.scalar.activation(act_sb[:], gin, AF.Gelu)
                            add_dep_helper(gi.ins, last_tbl.ins, sync=False,
                                           reason="act-table grouping")
                            for st in range(NST):
                                nc.sync.dma_start(
                                    a2a_in[ch][2 * st, pb],
                                    act_sb[0:64, st * H:(st + 1) * H])
                                nc.sync.dma_start(
                                    a2a_in[ch][2 * st + 1, pb],
                                    act_sb[64:128, st * H:(st + 1) * H])
                            prev_gelu = gi
                        pend = []
                        # fire this chunk's AllToAll (overlaps with compute)
                        nc.gpsimd.collective_compute(
                            "AllToAll", ALU.bypass, replica_groups=rg,
                            ins=[a2a_in[ch].opt()],
                            outs=[a2a_out[ch * 64:(ch + 1) * 64, :].opt()])

            # ---- phase 8: y_part[r, d] = sum_sh actT[sh, r] * owT[sh, d] ----
            # a2a_out rows are RS-order batches; XBAR transpose-DMA delivers
            # [128 sh, 256 r] stationary tiles directly.
            with (
                tc.tile_pool(name="p8a", bufs=6) as p8ap,
                tc.tile_pool(name="p8w", bufs=6) as p8wp,
                tc.tile_pool(name="ysb", bufs=2) as ysbp,
                tc.tile_pool(name="yps", bufs=2, space="PSUM") as yps,
            ):
                ypsum = []
                for _bt in range(2):
                    yp_t = yps.tile([128, D], F32, tag="yps")
                    ypsum.append(yp_t)
                for tcc in range(NC_T // 8):
                    at = p8ap.tile([128, 8 * 256], BF16, tag="at")
                    nc.scalar.dma_start_transpose(
                        at[:].rearrange("p (j b) -> p j b", j=8),
                        a2a_out[:, tcc * 1024:(tcc + 1) * 1024])
                    for cc in range(2 * tcc, 2 * tcc + 2):
                        ow_t = p8wp.tile([128, 4 * D], BF16, tag="ow")
                        nc.sync.dma_start(ow_t[:], owT[cc])
                        for j in range(4):
                            c = 4 * cc + j
                            jj = c - 8 * tcc
                            for bt in range(2):
                                nc.tensor.matmul(
                                    ypsum[bt][:],
                                    at[:, jj * 256 + bt * 128:
                                       jj * 256 + (bt + 1) * 128],
                                    ow_t[:, j * D:(j + 1) * D],
                                    start=(c == 0), stop=(c == NC_T - 1))
                for bt in range(2):
                    y_sb = ysbp.tile([128, D], F32, tag="ysb")
                    nc.vector.tensor_add(y_sb[:], ypsum[bt][:], outb_sb[:])
                    nc.sync.dma_start(y_part[bt * 128:(bt + 1) * 128, :],
                                      y_sb[:])

                nc.gpsimd.collective_compute(
                    "ReduceScatter", ALU.add, replica_groups=rg,
                    ins=[y_part.opt()], outs=[y_red.opt()])
                nc.sync.dma_start(y_out[:], y_red[:])

    nc.compile()
    return nc


_CACHE = {}


def _get_program(ln_trivial):
    if ln_trivial not in _CACHE:
        _CACHE[ln_trivial] = _build(ln_trivial)
    return _CACHE[ln_trivial]


def _prep_inputs(x, k_w, k_b, q_w, q_b, v_w, v_b, attn_bias, ln_g, ln_b,
                 out_w, out_b):
    ln_trivial = bool(np.all(ln_g == 1.0) and np.all(ln_b == 0.0))
    kq_wT = np.ascontiguousarray(
        np.concatenate([k_w.T, q_w.T], axis=1)).astype(BFNP).reshape(
            NDT, 128, 2 * H)
    v_wT = np.ascontiguousarray(v_w.T).astype(BFNP).reshape(NDT, 128, H)
    kqbb = np.tile(np.concatenate([k_b, q_b])[None, :],
                   (128, 2)).astype(BFNP)
    v_b2 = np.ascontiguousarray(v_b.reshape(NHT, 128, 1)).astype(np.float32)
    ab = np.ascontiguousarray(
        attn_bias.reshape(NST, 128, H).transpose(1, 0, 2).reshape(
            128, NST * H)).astype(BFNP)
    outb8 = np.ascontiguousarray(
        np.tile((out_b / N_CORES)[None, :], (128, 1))).astype(np.float32)
    owT_full = np.ascontiguousarray(out_w.T)  # [S*H, D]
    shared = dict(kq_wT=kq_wT, v_wT=v_wT, kqbb=kqbb, v_b2=v_b2, ab=ab,
                  outb8=outb8, ones128=np.ones((128, 128), BFNP))
    if not ln_trivial:
        shared["lng"] = np.tile(ln_g[None, :], (128, 1)).astype(BFNP)
        shared["lnb"] = np.tile(ln_b[None, :], (128, 1)).astype(BFNP)
    in_maps = []
    for i in range(N_CORES):
        xi = np.ascontiguousarray(
            x[i * NB:(i + 1) * NB].transpose(0, 2, 1)).astype(BFNP).reshape(
                NB, NDT, 128, S)
        owi = np.ascontiguousarray(
            owT_full[i * SLICE:(i + 1) * SLICE]).astype(BFNP).reshape(
                NC_T // 4, 4, 128, D).transpose(0, 2, 1, 3).reshape(
                NC_T // 4, 128, 4 * D)
        m = dict(shared)
        m["xT"] = xi
        m["owT"] = owi
        in_maps.append(m)
    return ln_trivial, in_maps


def _row_perm():
    """RS-row r -> global batch index. Row order is (chunk, src, bb)."""
    r = np.arange(N_CORES * NB)
    chk = r // (N_CORES * G)
    src = (r % (N_CORES * G)) // G
    bb = r % G
    return src * NB + chk * G + bb


def _gather(res):
    shards = [np.asarray(res.results[i]["y"], dtype=np.float32)
              for i in range(N_CORES)]
    ycat = np.concatenate(shards, axis=0)  # [256, 512] in RS row order
    y_full = np.empty_like(ycat)
    y_full[_row_perm()] = ycat
    return y_full


def kernel(**inputs):
    xs = {k: np.asarray(v, dtype=np.float32) for k, v in inputs.items()}
    ln_trivial, in_maps = _prep_inputs(
        xs["x"], xs["k_w"], xs["k_b"], xs["q_w"], xs["q_b"], xs["v_w"],
        xs["v_b"], xs["attn_bias"], xs["ln_g"], xs["ln_b"], xs["out_w"],
        xs["out_b"])
    nc = _get_program(ln_trivial)
    res = run_bass_kernel_spmd(nc, in_maps, core_ids=list(range(N_CORES)))
    return _gather(res).reshape(B, 1, D).astype(np.float32)
